# revision 11
# baseline (speedup 1.0000x reference)
"""BlockAttentionResidual routing kernel for 8 Trainium2 NeuronCores.

Computation (per token): S=9 sources (embedding + 8 block summaries),
RMS-normalized routing keys, dot with a static query -> softmax weights ->
weighted combine of raw sources + routing entropy.

Sharding: data-parallel over the flattened (B*T)=8192 token axis, 1024
tokens per core. Params (query*key_weight, identity) replicated.

Engine assignment per 128-token tile (9 sources x [128, 2048]):
  - ScalarE : Square+accum_out -> sum(src^2) for all 9 sources (rms),
              plus Copy+accum_out reduces for 2 of the gpsimd products
  - VectorE : fused scalar_tensor_tensor (src*qw, accum=sum) for 4 sources,
              tensor_scalar+accum reduces for 3 gpsimd products
  - GpSimd  : tensor_tensor src*qw -> bf16 product for 5 sources
  - TensorE : combine as 9 accumulating float32r matmuls with diagonal
              lhsT = diag(w_s) into PSUM (per-token scale + sum over s)
All ScalarE transcendentals (square/ln/exp/copy) live in the single
natural_log_exp_and_others table set; 1/sqrt(x) is exp(-0.5*ln(x)).
"""

import contextlib

import numpy as np

import concourse.bass as bass
import concourse.tile as tile
from concourse import bacc, mybir
from concourse import bass_utils

B, T, D, K = 4, 2048, 2048, 8
S = K + 1
N_CORES = 8
BT = B * T
TOK = BT // N_CORES          # tokens per core
P = 128                      # SBUF partitions / tokens per tile
NT = TOK // P                # token tiles per core
BANK = 512                   # fp32 elements per PSUM bank
NB = D // BANK
EPS = 1e-8
N_STT = 4                    # sources using fused DVE scalar_tensor_tensor
N_DVE_RED = 3                # gpsimd-product sources reduced on DVE

F32 = mybir.dt.float32
F32R = mybir.dt.float32r
BF16 = mybir.dt.bfloat16
ALU = mybir.AluOpType
ACT = mybir.ActivationFunctionType
AX = mybir.AxisListType

_nc_cache = []


def _build(repeat=1):
    nc = bacc.Bacc("TRN2", target_bir_lowering=False, debug=False,
                   num_devices=N_CORES)
    emb = nc.dram_tensor("emb", [TOK, D], F32R, kind="ExternalInput").ap()
    blk = nc.dram_tensor("blk", [K, TOK, D], F32R, kind="ExternalInput").ap()
    qw = nc.dram_tensor("qw", [P, D], F32, kind="ExternalInput").ap()
    ident = nc.dram_tensor("ident", [P, P], F32, kind="ExternalInput").ap()
    routed = nc.dram_tensor("routed", [TOK, D], F32, kind="ExternalOutput").ap()
    wout = nc.dram_tensor("wout", [TOK, S], F32, kind="ExternalOutput").ap()
    ent = nc.dram_tensor("ent", [TOK, 1], F32, kind="ExternalOutput").ap()

    with tile.TileContext(nc) as tc:
        with (
            tc.tile_pool(name="const", bufs=1) as const_pool,
            tc.tile_pool(name="src", bufs=2) as src_pool,
            tc.tile_pool(name="scr", bufs=1) as scr_pool,
            tc.tile_pool(name="gp", bufs=2) as gp_pool,
            tc.tile_pool(name="stat", bufs=2) as stat_pool,
            tc.tile_pool(name="diag", bufs=3) as diag_pool,
            tc.tile_pool(name="outp", bufs=1) as out_pool,
            tc.tile_pool(name="psum", bufs=2, space="PSUM") as psum_pool,
        ):
            qw_t = const_pool.tile([P, D], F32, tag="qw")
            nc.sync.dma_start(qw_t[:], qw[:, :])
            id_t = const_pool.tile([P, P], F32, tag="id")
            nc.sync.dma_start(id_t[:], ident[:, :])
            eps_t = const_pool.tile([P, 1], F32, tag="eps")
            nc.vector.memset(eps_t[:], EPS)

            loop_cm = (tc.For_i(0, repeat, 1) if repeat > 1
                       else contextlib.nullcontext())
            with loop_cm:
                for t in range(NT):
                    _tile_body(nc, t, emb, blk, routed, wout, ent,
                               qw_t, id_t, eps_t, src_pool, scr_pool,
                               gp_pool, stat_pool, diag_pool, out_pool,
                               psum_pool)

    nc.compile()
    return nc


def _tile_body(nc, t, emb, blk, routed, wout, ent, qw_t, id_t, eps_t,
               src_pool, scr_pool, gp_pool, stat_pool, diag_pool,
               out_pool, psum_pool):
    r0 = t * P
    # ---- load the 9 sources side by side in the free dim ----
    src = src_pool.tile([P, S * D], F32R, tag="src")
    nc.sync.dma_start(src[:, 0:D], emb[r0:r0 + P, :])
    nc.sync.dma_start(
        src[:, D:S * D].rearrange("p (k d) -> p k d", k=K),
        blk[:, r0:r0 + P, :].rearrange("k p d -> p k d"))

    # ---- per-source reductions over D ----
    ss = stat_pool.tile([P, S], F32, tag="ss")    # sum(src^2)
    uu = stat_pool.tile([P, S], F32, tag="uu")    # dot(src, qw)
    sq_scr = scr_pool.tile([P, D], BF16, tag="sq")
    tt_scr = scr_pool.tile([P, D], BF16, tag="tt")
    for s in range(S):
        sl = src[:, s * D:(s + 1) * D].bitcast(F32)
        nc.scalar.activation(sq_scr[:], sl, ACT.Square,
                             accum_out=ss[:, s:s + 1])
        if s < N_STT:
            nc.vector.scalar_tensor_tensor(
                out=tt_scr[:], in0=sl, scalar=1.0, in1=qw_t[:],
                op0=ALU.mult, op1=ALU.mult, accum_out=uu[:, s:s + 1])
        else:
            gp = gp_pool.tile([P, D], F32, tag="gp")
            nc.gpsimd.tensor_tensor(gp[:], sl, qw_t[:], ALU.mult)
            if s < N_STT + N_DVE_RED:
                nc.vector.tensor_scalar(
                    tt_scr[:], gp[:], 1.0, None, ALU.mult, ALU.add,
                    accum_out=uu[:, s:s + 1])
            else:
                nc.scalar.activation(sq_scr[:], gp[:], ACT.Copy,
                                     accum_out=uu[:, s:s + 1])

    # ---- logits = uu * rsqrt(ss/D + eps) via exp(-0.5*ln(.)) ----
    lnm = stat_pool.tile([P, S], F32, tag="lnm")
    nc.scalar.activation(lnm[:], ss[:], ACT.Ln, scale=1.0 / D, bias=eps_t[:])
    inv = stat_pool.tile([P, S], F32, tag="inv")
    nc.scalar.activation(inv[:], lnm[:], ACT.Exp, scale=-0.5)
    lg = stat_pool.tile([P, S], F32, tag="lg")
    nc.vector.tensor_tensor(lg[:], uu[:], inv[:], ALU.mult)

    # ---- softmax over the 9 sources ----
    mx = stat_pool.tile([P, 1], F32, tag="mx")
    nc.vector.tensor_reduce(mx[:], lg[:], AX.X, ALU.max)
    nmx = stat_pool.tile([P, 1], F32, tag="nmx")
    nc.vector.tensor_scalar(nmx[:], mx[:], -1.0, None, ALU.mult)
    e9 = stat_pool.tile([P, S], F32, tag="e9")
    s1 = stat_pool.tile([P, 1], F32, tag="s1")
    nc.scalar.activation(e9[:], lg[:], ACT.Exp, bias=nmx[:], scale=1.0,
                         accum_out=s1[:])
    rs = stat_pool.tile([P, 1], F32, tag="rs")
    nc.vector.reciprocal(rs[:], s1[:])
    w9 = stat_pool.tile([P, S], F32, tag="w9")
    nc.vector.tensor_scalar(w9[:], e9[:], rs[:], None, ALU.mult)
    nc.sync.dma_start(wout[r0:r0 + P, :], w9[:])

    # ---- entropy = ln(S) - sum(e*(lg-mx))/S ----
    x9 = stat_pool.tile([P, S], F32, tag="x9")
    nc.vector.tensor_scalar(x9[:], lg[:], nmx[:], None, ALU.add)
    ex_scr = stat_pool.tile([P, S], F32, tag="ex_scr")
    ex = stat_pool.tile([P, 1], F32, tag="ex")
    nc.vector.scalar_tensor_tensor(
        out=ex_scr[:], in0=e9[:], scalar=1.0, in1=x9[:],
        op0=ALU.mult, op1=ALU.mult, accum_out=ex[:])
    lns = stat_pool.tile([P, 1], F32, tag="lns")
    nc.scalar.activation(lns[:], s1[:], ACT.Ln)
    t1 = stat_pool.tile([P, 1], F32, tag="t1")
    nc.vector.tensor_tensor(t1[:], ex[:], rs[:], ALU.mult)
    et = stat_pool.tile([P, 1], F32, tag="et")
    nc.vector.tensor_tensor(et[:], lns[:], t1[:], ALU.subtract)
    nc.sync.dma_start(ent[r0:r0 + P, :], et[:])

    # ---- combine: routed = sum_s w_s * src_s via diag matmuls ----
    ps = psum_pool.tile([P, D], F32, tag="ps")
    for s in range(S):
        dg = diag_pool.tile([P, P], F32R, tag="dg")
        nc.vector.tensor_scalar(dg[:], id_t[:], w9[:, s:s + 1], None,
                                ALU.mult)
        for b in range(NB):
            nc.tensor.matmul(
                ps[:, b * BANK:(b + 1) * BANK],
                lhsT=dg[:],
                rhs=src[:, s * D + b * BANK:s * D + (b + 1) * BANK],
                start=(s == 0), stop=(s == S - 1))
    ot = out_pool.tile([P, D], F32, tag="ot")
    if t % 2 == 0:
        nc.scalar.copy(ot[:], ps[:])
    else:
        nc.vector.tensor_copy(ot[:], ps[:])
    nc.sync.dma_start(routed[r0:r0 + P, :], ot[:])


def _get_nc():
    if not _nc_cache:
        _nc_cache.append(_build())
    return _nc_cache[0]


def _run(embedding, blocks, query, key_weight, **spmd_kwargs):
    emb2 = np.ascontiguousarray(
        np.asarray(embedding, dtype=np.float32).reshape(BT, D))
    blk2 = np.asarray(blocks, dtype=np.float32).reshape(K, BT, D)
    qwv = (np.asarray(query, dtype=np.float32)
           * np.asarray(key_weight, dtype=np.float32))
    qw_b = np.ascontiguousarray(np.broadcast_to(qwv, (P, D)))
    idm = np.eye(P, dtype=np.float32)

    in_maps = []
    for c in range(N_CORES):
        sl = slice(c * TOK, (c + 1) * TOK)
        in_maps.append({
            "emb": emb2[sl],
            "blk": np.ascontiguousarray(blk2[:, sl, :]),
            "qw": qw_b,
            "ident": idm,
        })

    nc = _get_nc()
    res = bass_utils.run_bass_kernel_spmd(nc, in_maps,
                                          core_ids=list(range(N_CORES)),
                                          **spmd_kwargs)
    routed = np.concatenate(
        [res.results[c]["routed"] for c in range(N_CORES)], axis=0
    ).reshape(B, T, D)
    weights = np.concatenate(
        [res.results[c]["wout"] for c in range(N_CORES)], axis=0
    ).reshape(B, T, S)
    entropy = np.concatenate(
        [res.results[c]["ent"] for c in range(N_CORES)], axis=0
    ).reshape(B, T)
    return (routed, weights, entropy), res


def kernel(embedding, blocks, query, key_weight):
    outs, _ = _run(embedding, blocks, query, key_weight)
    return outs


# revision 12
# speedup vs baseline: 1.2123x; 1.2123x over previous
"""BlockAttentionResidual routing kernel for 8 Trainium2 NeuronCores.

Computation (per token): S=9 sources (embedding + 8 block summaries),
RMS-normalized routing keys, dot with a static query -> softmax weights ->
weighted combine of raw sources + routing entropy.

Sharding: data-parallel over the flattened (B*T)=8192 token axis, 1024
tokens per core. Params (query*key_weight, identity) replicated.

Engine assignment per 128-token tile (9 sources x [128, 2048]):
  - ScalarE : Square+accum_out -> sum(src^2) for all 9 sources (rms),
              plus Copy+accum_out reduces for 2 of the gpsimd products
  - VectorE : fused scalar_tensor_tensor (src*qw, accum=sum) for 4 sources,
              tensor_scalar+accum reduces for 3 gpsimd products
  - GpSimd  : tensor_tensor src*qw -> bf16 product for 5 sources
  - TensorE : combine as 9 accumulating float32r matmuls with diagonal
              lhsT = diag(w_s) into PSUM (per-token scale + sum over s)
All ScalarE transcendentals (square/ln/exp/copy) live in the single
natural_log_exp_and_others table set; 1/sqrt(x) is exp(-0.5*ln(x)).
"""

import contextlib

import numpy as np

import concourse.bass as bass
import concourse.tile as tile
from concourse import bacc, mybir
from concourse import bass_utils

B, T, D, K = 4, 2048, 2048, 8
S = K + 1
N_CORES = 8
BT = B * T
TOK = BT // N_CORES          # tokens per core
P = 128                      # SBUF partitions / tokens per tile
NT = TOK // P                # token tiles per core
BANK = 512                   # fp32 elements per PSUM bank
NB = D // BANK
EPS = 1e-8
N_STT = 9                    # sources using fused DVE scalar_tensor_tensor
N_DVE_RED = 3                # gpsimd-product sources reduced on DVE

F32 = mybir.dt.float32
F32R = mybir.dt.float32r
BF16 = mybir.dt.bfloat16
ALU = mybir.AluOpType
ACT = mybir.ActivationFunctionType
AX = mybir.AxisListType

_nc_cache = []


def _build(repeat=1):
    nc = bacc.Bacc("TRN2", target_bir_lowering=False, debug=False,
                   num_devices=N_CORES)
    emb = nc.dram_tensor("emb", [TOK, D], F32R, kind="ExternalInput").ap()
    blk = nc.dram_tensor("blk", [K, TOK, D], F32R, kind="ExternalInput").ap()
    qw = nc.dram_tensor("qw", [P, D], F32, kind="ExternalInput").ap()
    ident = nc.dram_tensor("ident", [P, P], F32, kind="ExternalInput").ap()
    routed = nc.dram_tensor("routed", [TOK, D], F32, kind="ExternalOutput").ap()
    wout = nc.dram_tensor("wout", [TOK, S], F32, kind="ExternalOutput").ap()
    ent = nc.dram_tensor("ent", [TOK, 1], F32, kind="ExternalOutput").ap()

    with tile.TileContext(nc) as tc:
        with (
            tc.tile_pool(name="const", bufs=1) as const_pool,
            tc.tile_pool(name="src", bufs=2) as src_pool,
            tc.tile_pool(name="scr", bufs=1) as scr_pool,
            tc.tile_pool(name="gp", bufs=2) as gp_pool,
            tc.tile_pool(name="stat", bufs=2) as stat_pool,
            tc.tile_pool(name="diag", bufs=3) as diag_pool,
            tc.tile_pool(name="outp", bufs=2) as out_pool,
            tc.tile_pool(name="psum", bufs=2, space="PSUM") as psum_pool,
        ):
            qw_t = const_pool.tile([P, D], F32, tag="qw")
            nc.sync.dma_start(qw_t[:], qw[:, :])
            id_t = const_pool.tile([P, P], F32, tag="id")
            nc.sync.dma_start(id_t[:], ident[:, :])
            eps_t = const_pool.tile([P, 1], F32, tag="eps")
            nc.vector.memset(eps_t[:], EPS)

            loop_cm = (tc.For_i(0, repeat, 1) if repeat > 1
                       else contextlib.nullcontext())
            with loop_cm:
                for t in range(NT):
                    _tile_body(nc, t, emb, blk, routed, wout, ent,
                               qw_t, id_t, eps_t, src_pool, scr_pool,
                               gp_pool, stat_pool, diag_pool, out_pool,
                               psum_pool)

    nc.compile()
    return nc


def _tile_body(nc, t, emb, blk, routed, wout, ent, qw_t, id_t, eps_t,
               src_pool, scr_pool, gp_pool, stat_pool, diag_pool,
               out_pool, psum_pool):
    r0 = t * P
    # ---- load the 9 sources side by side in the free dim ----
    src = src_pool.tile([P, S * D], F32R, tag="src")
    nc.sync.dma_start(src[:, 0:D], emb[r0:r0 + P, :])
    nc.sync.dma_start(
        src[:, D:S * D].rearrange("p (k d) -> p k d", k=K),
        blk[:, r0:r0 + P, :].rearrange("k p d -> p k d"))

    # ---- per-source reductions over D ----
    ss = stat_pool.tile([P, S], F32, tag="ss")    # sum(src^2)
    uu = stat_pool.tile([P, S], F32, tag="uu")    # dot(src, qw)
    sq_scr = scr_pool.tile([P, D], BF16, tag="sq")
    tt_scr = scr_pool.tile([P, D], BF16, tag="tt")
    for s in range(S):
        sl = src[:, s * D:(s + 1) * D].bitcast(F32)
        nc.scalar.activation(sq_scr[:], sl, ACT.Square,
                             accum_out=ss[:, s:s + 1])
        if s < N_STT:
            nc.vector.scalar_tensor_tensor(
                out=tt_scr[:], in0=sl, scalar=1.0, in1=qw_t[:],
                op0=ALU.mult, op1=ALU.mult, accum_out=uu[:, s:s + 1])
        else:
            gp = gp_pool.tile([P, D], F32, tag="gp")
            nc.gpsimd.tensor_tensor(gp[:], sl, qw_t[:], ALU.mult)
            if s < N_STT + N_DVE_RED:
                nc.vector.tensor_scalar(
                    tt_scr[:], gp[:], 1.0, None, ALU.mult, ALU.add,
                    accum_out=uu[:, s:s + 1])
            else:
                nc.scalar.activation(sq_scr[:], gp[:], ACT.Copy,
                                     accum_out=uu[:, s:s + 1])

    # ---- logits = uu * rsqrt(ss/D + eps) via exp(-0.5*ln(.)) ----
    lnm = stat_pool.tile([P, S], F32, tag="lnm")
    nc.scalar.activation(lnm[:], ss[:], ACT.Ln, scale=1.0 / D, bias=eps_t[:])
    inv = stat_pool.tile([P, S], F32, tag="inv")
    nc.scalar.activation(inv[:], lnm[:], ACT.Exp, scale=-0.5)
    lg = stat_pool.tile([P, S], F32, tag="lg")
    nc.vector.tensor_tensor(lg[:], uu[:], inv[:], ALU.mult)

    # ---- softmax over the 9 sources ----
    mx = stat_pool.tile([P, 1], F32, tag="mx")
    nc.vector.tensor_reduce(mx[:], lg[:], AX.X, ALU.max)
    nmx = stat_pool.tile([P, 1], F32, tag="nmx")
    nc.vector.tensor_scalar(nmx[:], mx[:], -1.0, None, ALU.mult)
    e9 = stat_pool.tile([P, S], F32, tag="e9")
    s1 = stat_pool.tile([P, 1], F32, tag="s1")
    nc.scalar.activation(e9[:], lg[:], ACT.Exp, bias=nmx[:], scale=1.0,
                         accum_out=s1[:])
    rs = stat_pool.tile([P, 1], F32, tag="rs")
    nc.vector.reciprocal(rs[:], s1[:])
    w9 = stat_pool.tile([P, S], F32, tag="w9")
    nc.vector.tensor_scalar(w9[:], e9[:], rs[:], None, ALU.mult)
    nc.sync.dma_start(wout[r0:r0 + P, :], w9[:])

    # ---- entropy = ln(S) - sum(e*(lg-mx))/S ----
    x9 = stat_pool.tile([P, S], F32, tag="x9")
    nc.vector.tensor_scalar(x9[:], lg[:], nmx[:], None, ALU.add)
    ex_scr = stat_pool.tile([P, S], F32, tag="ex_scr")
    ex = stat_pool.tile([P, 1], F32, tag="ex")
    nc.vector.scalar_tensor_tensor(
        out=ex_scr[:], in0=e9[:], scalar=1.0, in1=x9[:],
        op0=ALU.mult, op1=ALU.mult, accum_out=ex[:])
    lns = stat_pool.tile([P, 1], F32, tag="lns")
    nc.scalar.activation(lns[:], s1[:], ACT.Ln)
    t1 = stat_pool.tile([P, 1], F32, tag="t1")
    nc.vector.tensor_tensor(t1[:], ex[:], rs[:], ALU.mult)
    et = stat_pool.tile([P, 1], F32, tag="et")
    nc.vector.tensor_tensor(et[:], lns[:], t1[:], ALU.subtract)
    nc.sync.dma_start(ent[r0:r0 + P, :], et[:])

    # ---- combine: routed = sum_s w_s * src_s via diag matmuls ----
    ps = psum_pool.tile([P, D], F32, tag="ps")
    for s in range(S):
        dg = diag_pool.tile([P, P], F32R, tag="dg")
        nc.vector.tensor_scalar(dg[:], id_t[:], w9[:, s:s + 1], None,
                                ALU.mult)
        for b in range(NB):
            nc.tensor.matmul(
                ps[:, b * BANK:(b + 1) * BANK],
                lhsT=dg[:],
                rhs=src[:, s * D + b * BANK:s * D + (b + 1) * BANK],
                start=(s == 0), stop=(s == S - 1))
    ot = out_pool.tile([P, D], F32, tag="ot")
    if t % 2 == 0:
        nc.scalar.copy(ot[:], ps[:])
    else:
        nc.vector.tensor_copy(ot[:], ps[:])
    nc.sync.dma_start(routed[r0:r0 + P, :], ot[:])


def _get_nc():
    if not _nc_cache:
        _nc_cache.append(_build())
    return _nc_cache[0]


def _run(embedding, blocks, query, key_weight, **spmd_kwargs):
    emb2 = np.ascontiguousarray(
        np.asarray(embedding, dtype=np.float32).reshape(BT, D))
    blk2 = np.asarray(blocks, dtype=np.float32).reshape(K, BT, D)
    qwv = (np.asarray(query, dtype=np.float32)
           * np.asarray(key_weight, dtype=np.float32))
    qw_b = np.ascontiguousarray(np.broadcast_to(qwv, (P, D)))
    idm = np.eye(P, dtype=np.float32)

    in_maps = []
    for c in range(N_CORES):
        sl = slice(c * TOK, (c + 1) * TOK)
        in_maps.append({
            "emb": emb2[sl],
            "blk": np.ascontiguousarray(blk2[:, sl, :]),
            "qw": qw_b,
            "ident": idm,
        })

    nc = _get_nc()
    res = bass_utils.run_bass_kernel_spmd(nc, in_maps,
                                          core_ids=list(range(N_CORES)),
                                          **spmd_kwargs)
    routed = np.concatenate(
        [res.results[c]["routed"] for c in range(N_CORES)], axis=0
    ).reshape(B, T, D)
    weights = np.concatenate(
        [res.results[c]["wout"] for c in range(N_CORES)], axis=0
    ).reshape(B, T, S)
    entropy = np.concatenate(
        [res.results[c]["ent"] for c in range(N_CORES)], axis=0
    ).reshape(B, T)
    return (routed, weights, entropy), res


def kernel(embedding, blocks, query, key_weight):
    outs, _ = _run(embedding, blocks, query, key_weight)
    return outs


# revision 13
# speedup vs baseline: 1.2345x; 1.0183x over previous
"""BlockAttentionResidual routing kernel for 8 Trainium2 NeuronCores.

Computation (per token): S=9 sources (embedding + 8 block summaries),
RMS-normalized routing keys, dot with a static query -> softmax weights ->
weighted combine of raw sources + routing entropy.

Sharding: data-parallel over the flattened (B*T)=8192 token axis, 1024
tokens per core. Params (query*key_weight, identity) replicated.

Engine assignment per 128-token tile (9 sources x [128, 2048]):
  - ScalarE : Square activation with fused accum_out -> sum(src^2) (rms)
  - VectorE : fused scalar_tensor_tensor (src*qw, accum=sum) -> dots
  - TensorE : combine as 9 accumulating float32r matmuls with diagonal
              lhsT = diag(w_s) into PSUM (per-token scale + sum over s);
              the PSUM->SBUF copy alternates ScalarE/VectorE per tile
All ScalarE transcendentals (square/ln/exp/copy) live in the single
natural_log_exp_and_others table set; 1/sqrt(x) is exp(-0.5*ln(x)).
"""

import contextlib

import numpy as np

import concourse.bass as bass
import concourse.tile as tile
from concourse import bacc, mybir
from concourse import bass_utils

B, T, D, K = 4, 2048, 2048, 8
S = K + 1
N_CORES = 8
BT = B * T
TOK = BT // N_CORES          # tokens per core
P = 128                      # SBUF partitions / tokens per tile
NT = TOK // P                # token tiles per core
BANK = 512                   # fp32 elements per PSUM bank
NB = D // BANK
EPS = 1e-8
N_STT = 9                    # sources using fused DVE scalar_tensor_tensor
N_DVE_RED = 3                # gpsimd-product sources reduced on DVE

F32 = mybir.dt.float32
F32R = mybir.dt.float32r
BF16 = mybir.dt.bfloat16
ALU = mybir.AluOpType
ACT = mybir.ActivationFunctionType
AX = mybir.AxisListType

_nc_cache = []


def _build(repeat=1):
    nc = bacc.Bacc("TRN2", target_bir_lowering=False, debug=False,
                   num_devices=N_CORES)
    emb = nc.dram_tensor("emb", [TOK, D], F32R, kind="ExternalInput").ap()
    blk = nc.dram_tensor("blk", [K, TOK, D], F32R, kind="ExternalInput").ap()
    qw = nc.dram_tensor("qw", [P, D], F32, kind="ExternalInput").ap()
    ident = nc.dram_tensor("ident", [P, P], F32, kind="ExternalInput").ap()
    routed = nc.dram_tensor("routed", [TOK, D], F32, kind="ExternalOutput").ap()
    wout = nc.dram_tensor("wout", [TOK, S], F32, kind="ExternalOutput").ap()
    ent = nc.dram_tensor("ent", [TOK, 1], F32, kind="ExternalOutput").ap()

    with tile.TileContext(nc) as tc:
        with (
            tc.tile_pool(name="const", bufs=1) as const_pool,
            tc.tile_pool(name="src", bufs=2) as src_pool,
            tc.tile_pool(name="scr", bufs=1) as scr_pool,
            tc.tile_pool(name="gp", bufs=2) as gp_pool,
            tc.tile_pool(name="stat", bufs=2) as stat_pool,
            tc.tile_pool(name="diag", bufs=3) as diag_pool,
            tc.tile_pool(name="outp", bufs=2) as out_pool,
            tc.tile_pool(name="psum", bufs=2, space="PSUM") as psum_pool,
        ):
            qw_t = const_pool.tile([P, D], F32, tag="qw")
            nc.sync.dma_start(qw_t[:], qw[:, :])
            id_t = const_pool.tile([P, P], F32, tag="id")
            nc.sync.dma_start(id_t[:], ident[:, :])
            eps_t = const_pool.tile([P, 1], F32, tag="eps")
            nc.vector.memset(eps_t[:], EPS)

            loop_cm = (tc.For_i(0, repeat, 1) if repeat > 1
                       else contextlib.nullcontext())
            with loop_cm:
                for t in range(NT):
                    _tile_body(nc, t, emb, blk, routed, wout, ent,
                               qw_t, id_t, eps_t, src_pool, scr_pool,
                               gp_pool, stat_pool, diag_pool, out_pool,
                               psum_pool)

    nc.compile()
    return nc


def _tile_body(nc, t, emb, blk, routed, wout, ent, qw_t, id_t, eps_t,
               src_pool, scr_pool, gp_pool, stat_pool, diag_pool,
               out_pool, psum_pool):
    r0 = t * P
    # ---- load the 9 sources side by side in the free dim ----
    src = src_pool.tile([P, S * D], F32R, tag="src")
    nc.sync.dma_start(src[:, 0:D], emb[r0:r0 + P, :])
    nc.sync.dma_start(
        src[:, D:S * D].rearrange("p (k d) -> p k d", k=K),
        blk[:, r0:r0 + P, :].rearrange("k p d -> p k d"))

    # ---- per-source reductions over D ----
    ss = stat_pool.tile([P, S], F32, tag="ss")    # sum(src^2)
    uu = stat_pool.tile([P, S], F32, tag="uu")    # dot(src, qw)
    sq_scr = scr_pool.tile([P, D], BF16, tag="sq")
    tt_scr = scr_pool.tile([P, D], BF16, tag="tt")
    for s in range(S):
        sl = src[:, s * D:(s + 1) * D].bitcast(F32)
        nc.scalar.activation(sq_scr[:], sl, ACT.Square,
                             accum_out=ss[:, s:s + 1])
        if s < N_STT:
            nc.vector.scalar_tensor_tensor(
                out=tt_scr[:], in0=sl, scalar=1.0, in1=qw_t[:],
                op0=ALU.mult, op1=ALU.mult, accum_out=uu[:, s:s + 1])
        else:
            gp = gp_pool.tile([P, D], F32, tag="gp")
            nc.gpsimd.tensor_tensor(gp[:], sl, qw_t[:], ALU.mult)
            if s < N_STT + N_DVE_RED:
                nc.vector.tensor_scalar(
                    tt_scr[:], gp[:], 1.0, None, ALU.mult, ALU.add,
                    accum_out=uu[:, s:s + 1])
            else:
                nc.scalar.activation(sq_scr[:], gp[:], ACT.Copy,
                                     accum_out=uu[:, s:s + 1])

    # ---- logits = uu * rsqrt(ss/D + eps) via exp(-0.5*ln(.)) ----
    lnm = stat_pool.tile([P, S], F32, tag="lnm")
    nc.scalar.activation(lnm[:], ss[:], ACT.Ln, scale=1.0 / D, bias=eps_t[:])
    inv = stat_pool.tile([P, S], F32, tag="inv")
    nc.scalar.activation(inv[:], lnm[:], ACT.Exp, scale=-0.5)
    lg = stat_pool.tile([P, S], F32, tag="lg")
    nc.vector.tensor_tensor(lg[:], uu[:], inv[:], ALU.mult)

    # ---- softmax over the 9 sources ----
    mx = stat_pool.tile([P, 1], F32, tag="mx")
    nc.vector.tensor_reduce(mx[:], lg[:], AX.X, ALU.max)
    nmx = stat_pool.tile([P, 1], F32, tag="nmx")
    nc.vector.tensor_scalar(nmx[:], mx[:], -1.0, None, ALU.mult)
    e9 = stat_pool.tile([P, S], F32, tag="e9")
    s1 = stat_pool.tile([P, 1], F32, tag="s1")
    nc.scalar.activation(e9[:], lg[:], ACT.Exp, bias=nmx[:], scale=1.0,
                         accum_out=s1[:])
    rs = stat_pool.tile([P, 1], F32, tag="rs")
    nc.vector.reciprocal(rs[:], s1[:])
    w9 = stat_pool.tile([P, S], F32, tag="w9")
    nc.vector.tensor_scalar(w9[:], e9[:], rs[:], None, ALU.mult)
    nc.sync.dma_start(wout[r0:r0 + P, :], w9[:])

    # ---- entropy = ln(S) - sum(e*(lg-mx))/S ----
    x9 = stat_pool.tile([P, S], F32, tag="x9")
    nc.vector.tensor_scalar(x9[:], lg[:], nmx[:], None, ALU.add)
    ex_scr = stat_pool.tile([P, S], F32, tag="ex_scr")
    ex = stat_pool.tile([P, 1], F32, tag="ex")
    nc.vector.scalar_tensor_tensor(
        out=ex_scr[:], in0=e9[:], scalar=1.0, in1=x9[:],
        op0=ALU.mult, op1=ALU.mult, accum_out=ex[:])
    lns = stat_pool.tile([P, 1], F32, tag="lns")
    nc.scalar.activation(lns[:], s1[:], ACT.Ln)
    t1 = stat_pool.tile([P, 1], F32, tag="t1")
    nc.vector.tensor_tensor(t1[:], ex[:], rs[:], ALU.mult)
    et = stat_pool.tile([P, 1], F32, tag="et")
    nc.vector.tensor_tensor(et[:], lns[:], t1[:], ALU.subtract)
    nc.sync.dma_start(ent[r0:r0 + P, :], et[:])

    # ---- combine: routed = sum_s w_s * src_s via diag matmuls ----
    ps = psum_pool.tile([P, D], F32, tag="ps")
    for s in range(S):
        dg = diag_pool.tile([P, P], F32R, tag="dg")
        nc.vector.tensor_scalar(dg[:], id_t[:], w9[:, s:s + 1], None,
                                ALU.mult)
        for b in range(NB):
            nc.tensor.matmul(
                ps[:, b * BANK:(b + 1) * BANK],
                lhsT=dg[:],
                rhs=src[:, s * D + b * BANK:s * D + (b + 1) * BANK],
                start=(s == 0), stop=(s == S - 1))
    ot = out_pool.tile([P, D], F32, tag="ot")
    if t % 2 == 0:
        nc.scalar.copy(ot[:], ps[:])
    else:
        nc.vector.tensor_copy(ot[:], ps[:])
    nc.sync.dma_start(routed[r0:r0 + P, :], ot[:])


def _get_nc():
    if not _nc_cache:
        _nc_cache.append(_build())
    return _nc_cache[0]


def _run(embedding, blocks, query, key_weight, **spmd_kwargs):
    emb2 = np.ascontiguousarray(
        np.asarray(embedding, dtype=np.float32).reshape(BT, D))
    blk2 = np.asarray(blocks, dtype=np.float32).reshape(K, BT, D)
    qwv = (np.asarray(query, dtype=np.float32)
           * np.asarray(key_weight, dtype=np.float32))
    qw_b = np.ascontiguousarray(np.broadcast_to(qwv, (P, D)))
    idm = np.eye(P, dtype=np.float32)

    in_maps = []
    for c in range(N_CORES):
        sl = slice(c * TOK, (c + 1) * TOK)
        in_maps.append({
            "emb": emb2[sl],
            "blk": np.ascontiguousarray(blk2[:, sl, :]),
            "qw": qw_b,
            "ident": idm,
        })

    nc = _get_nc()
    res = bass_utils.run_bass_kernel_spmd(nc, in_maps,
                                          core_ids=list(range(N_CORES)),
                                          **spmd_kwargs)
    routed = np.concatenate(
        [res.results[c]["routed"] for c in range(N_CORES)], axis=0
    ).reshape(B, T, D)
    weights = np.concatenate(
        [res.results[c]["wout"] for c in range(N_CORES)], axis=0
    ).reshape(B, T, S)
    entropy = np.concatenate(
        [res.results[c]["ent"] for c in range(N_CORES)], axis=0
    ).reshape(B, T)
    return (routed, weights, entropy), res


def kernel(embedding, blocks, query, key_weight):
    outs, _ = _run(embedding, blocks, query, key_weight)
    return outs
